# revision 32
# baseline (speedup 1.0000x reference)
"""Trainium2 Bass kernel for the 5x5-neighborhood min-L1 loss (nn_NNLoss).

Computation (faithful to the reference):
    gt_pad = pad(ground_truth, rows by nw//2, cols by nh//2, value=-10000)
    norms[b,h,w,s] = sum_c |gt_pad[b,c,h+di,w+dj] - pred[b,c,h,w]|
                     for s=(di,dj), di in range(nh), dj in range(nw)
    loss = mean over (b,h,w) of min_s norms

Sharding: pure data parallel over the batch dim: 16 images -> 2 per core
across 8 NeuronCores.  Each core returns per-partition partial sums
[128,1]; the host adds them up and divides (the scalar "all-reduce").

Per-core layout (v5 -- single row-block, 2 rows per partition):
  - partition p holds image rows {2p, 2p+1} (sub-row s in {0,1}); free
    dim is [q=(img,chan), s, w].  Every HBM load is ONE dma with 2KB
    contiguous descriptors and the whole H=256 fits one partition block.
  - ground_truth is loaded ONCE via a SWDGE dma that casts f32->bf16 in
    flight (descriptor emission finishes before compute starts, so the
    Q7 ring traffic never contends with DVE); the nh row shifts
    decompose into partition shifts k in {-1,0,+1} (built from the base
    tile by two SBUF->SBUF DMAs on the otherwise-idle sync HWDGE queue)
    plus a sub-row select s'.
  - NO pad values are materialized: out-of-range column shifts are
    excluded from the running min by restricting the min-update APs to
    the valid w range, and out-of-range rows by memsetting the boundary
    partitions of the shifted tiles to +10000 (|10000 - pred| can never
    win the min: real sums are < ~30).  All memsets run on gpsimd.
  - per (di, s) unit: one wide sub (DVE, all nw column shifts via an
    overlapping-window AP at 2x bf16) -> |.| in place (ACT) -> channel
    sum (2 DVE adds) -> w-restricted running-min updates into a single
    shared m tile [i][s][w], reduced once at the end.
"""

import os

# The execution path needs the axon PJRT platform; a harness that pins
# JAX_PLATFORMS=cpu would hide the NeuronCores from jax.
if "axon" not in os.environ.get("JAX_PLATFORMS", "axon"):
    os.environ.pop("JAX_PLATFORMS", None)

import numpy as np

B, C, H, W = 16, 3, 256, 256
N_CORES = 8
IPC = B // N_CORES  # images per core
PAD_BIG = 10000.0  # stand-in for the reference's pad: never wins the min

_BUILD_CACHE = {}
LAST_EXEC_NS = [None]  # exec_time_ns of the last traced run (for test.py)
LAST_RES = [None]  # full BassKernelResults of the last run (for analysis)


def _build(nh, nw):
    """Trace the Bass/Tile program for one core. Returns the Bass object."""
    from contextlib import ExitStack

    import concourse.bacc as bacc
    import concourse.bass as bass  # noqa: F401
    import concourse.tile as tile
    from concourse import mybir
    from concourse.alu_op_type import AluOpType

    f32 = mybir.dt.float32
    # bf16, not fp16: the DVE's 2x tensor_tensor packing mode only has
    # uops for bf16 (fp16 measured at 1x on HW)
    f16 = mybir.dt.bfloat16
    Abs = mybir.ActivationFunctionType.Abs
    Copy = mybir.ActivationFunctionType.Copy

    # Faithful to the reference's crossed pad/shift pairing:
    #   row shifts   di in range(nh), offset d  = di - nw//2
    #   col shifts   g  in range(nw), offset    = g  - nh//2
    H_PAD = nw // 2
    W_PAD = nh // 2
    NDI, G = nh, nw
    S = 2  # rows packed per partition
    assert H == 128 * S
    Q = C * IPC  # fused (img, chan) chunks: 6
    SW = S * W  # 512
    FDW = Q * SW  # 3072: data columns of the packed tiles
    MARG = W_PAD  # margin columns so the window AP stays in-bounds
    GQW = G * Q * W  # 7680: one (di, s) diff tensor [g][q][w]
    GIW = G * IPC * W  # 2560: one (di, s) channel-summed tensor [g][i][w]
    IW = IPC * SW  # 1024: running-min tile [i][s][w]

    # (di, s) -> (partition shift k, source sub-row s'): the target row
    # 2p + s + (di - H_PAD) lives at partition p + k, sub-row s'
    def shift_of(di, s):
        idx = s + di - H_PAD
        return idx // S, idx % S

    ks_needed = sorted(
        {shift_of(di, s)[0] for di in range(NDI) for s in range(S)}
    )
    # process row shifts that only need the unshifted tile first: the PE
    # builds the shifted tiles (~7us) while the first subs run
    dis = sorted(
        range(NDI), key=lambda di: max(abs(shift_of(di, s)[0]) for s in (0, 1))
    )

    # valid output-w range for column shift g (shifts reading outside the
    # row are excluded from the min -- the reference's pad value loses
    # every min it enters, so exclusion is equivalent)
    def wrange(g):
        lo = max(0, W_PAD - g)
        hi = W + min(0, W_PAD - g)
        return lo, hi

    # Bacc (not raw Bass): its compile() splits multi-wait instructions
    # (TRN2 allows at most one sync wait per instruction) among other
    # required lowerings.
    nc = bacc.Bacc("TRN2", target_bir_lowering=False, debug=False)
    pred_d = nc.dram_tensor("predicted", [IPC, C, H, W], f32, kind="ExternalInput")
    gt_d = nc.dram_tensor("ground_truth", [IPC, C, H, W], f32, kind="ExternalInput")
    # stacked shifted identities [k-index, 128, 128] for the PE-based
    # partition shifts (lhsT[k, p] = 1 iff k = p + shift)
    n_eyes = len([k for k in ks_needed if k != 0])
    eye_d = nc.dram_tensor("shifteye", [128, n_eyes * 128], f16, kind="ExternalInput")
    out_d = nc.dram_tensor("partials", [128, 1], f32, kind="ExternalOutput")

    import bass_rust as _br

    def strided(ap, levels, extra_offset=0):
        """Hand-built free-dim AP on an existing [128, N] view (keeps the
        partition level and base offset)."""
        c = ap.copy()
        c.ap = _br.VecI64Pair([list(ap.ap[0])] + [list(l) for l in levels])
        if extra_offset:
            c.offset = c.offset + extra_offset
        return c

    with tile.TileContext(nc) as tc, ExitStack() as ctx:
        g_pool = ctx.enter_context(tc.tile_pool(name="gt", bufs=1))
        p_pool = ctx.enter_context(tc.tile_pool(name="pred", bufs=1))
        d_pool = ctx.enter_context(tc.tile_pool(name="d", bufs=4))
        s_pool = ctx.enter_context(tc.tile_pool(name="s", bufs=3))
        m_pool = ctx.enter_context(tc.tile_pool(name="m", bufs=1))
        r_pool = ctx.enter_context(tc.tile_pool(name="r", bufs=1))

        # ---- ground truth: one SWDGE dma, f32->bf16 cast in flight,
        # 2KB descriptors (2 contiguous rows per partition).  Both input
        # loads together read 3.1MB -- the ~9us load phase is HBM-bound
        # either way, and the in-flight cast avoids a separate cast op ----
        gt_t = {}
        gt_t[0] = g_pool.tile(
            [128, MARG + FDW + MARG], f16, tag="gt0", name="gt0"
        )
        nc.gpsimd.memset(gt_t[0][:, 0:MARG], PAD_BIG)
        nc.gpsimd.memset(gt_t[0][:, MARG + FDW :], PAD_BIG)
        nc.gpsimd.dma_start(
            gt_t[0][:, MARG : MARG + FDW].rearrange("p (q x) -> p q x", q=Q),
            gt_d.ap().rearrange("i c (p s) w -> p (i c) (s w)", s=S),
        )

        # ---- predicted: HWDGE f32 load on the ACT queue + ACT cast ----
        p_stage = p_pool.tile([128, FDW], f32, tag="p_stage", name="p_stage")
        nc.scalar.dma_start(
            p_stage.rearrange("p (q x) -> p q x", q=Q),
            pred_d.ap().rearrange("i c (p s) w -> p (i c) (s w)", s=S),
        )
        pred_t = p_pool.tile([128, FDW], f16, tag="pred", name="pred")
        nc.scalar.activation(pred_t[:, :], p_stage[:, :], Copy)

        # ---- partition-shifted gt copies, built ON-CHIP by the (idle)
        # TensorEngine: matmul with a shifted identity moves partition
        # p+k -> p exactly (bf16 x {0,1} is lossless), landing in PSUM
        # f32; one ACT op casts PSUM -> bf16 SBUF.  SBUF->SBUF DMA
        # measured 5-23 GB/s (60us+ per shift) so DMA is not an option.
        # The base tile's PAD_BIG margins shift along with the data; the
        # boundary partition (no source row) comes out 0 and is patched
        # to PAD_BIG by a tiny one-partition DMA from a const tile. ----
        WTOT = MARG + FDW + MARG
        eye_t = g_pool.tile([128, n_eyes * 128], f16, tag="eye", name="eye_t")
        nc.scalar.dma_start(eye_t[:, :], eye_d.ap())
        cpad = g_pool.tile([32, WTOT], f16, tag="cpad", name="cpad")
        nc.gpsimd.memset(cpad[:, :], PAD_BIG)
        ps_pool = ctx.enter_context(tc.tile_pool(name="ps", bufs=1, space="PSUM"))
        for ei, k in enumerate([k for k in ks_needed if k != 0]):
            t = g_pool.tile([128, WTOT], f16, tag=f"gt{k}", name=f"gt{k}")
            ps = ps_pool.tile([128, WTOT], f32, tag="ps", name=f"ps{k}")
            lhsT = eye_t[:, ei * 128 : (ei + 1) * 128]
            for c in range(0, WTOT, 512):
                wid = min(512, WTOT - c)
                nc.tensor.matmul(
                    ps[:, c : c + wid],
                    lhsT,
                    gt_t[0][:, c : c + wid],
                    start=True,
                    stop=True,
                )
            nc.scalar.activation(t[:, :], ps[:, :], Copy)
            bp = 0 if k < 0 else 127
            nc.sync.dma_start(t[bp : bp + 1, :], cpad[0:1, :])
            gt_t[k] = t
        nc._shift_ks = [k for k in ks_needed if k != 0]

        GQW2 = G * Q * SW  # 15360: paired diff tile [g][q][s][w]
        GIW2 = G * IPC * SW  # 5120: paired channel-sum tile [g][i][s][w]
        m = None
        for di in dis:
            # ---- two wide subs (one per packed sub-row s), all G column
            # shifts each, into one shared diff tile [g][q][s][w]: the
            # pairing halves the op count of everything downstream ----
            d = d_pool.tile([128, GQW2], f16, tag="d", name=f"d{di}", bufs=2)
            for s in (0, 1):
                k, sp = shift_of(di, s)
                gt_op = strided(
                    gt_t[k][:, :], [[1, G], [SW, Q], [1, W]], MARG + sp * W - W_PAD
                )
                pr_op = strided(pred_t[:, :], [[0, G], [SW, Q], [1, W]], s * W)
                d_out = strided(d[:, :], [[Q * SW, G], [SW, Q], [1, W]], s * W)
                nc.vector.tensor_sub(d_out, gt_op, pr_op)

            # ---- ONE |d| per pair on ACT (ACT has ~1.4us fixed cost
            # per op; rate is ~1 elem/cycle either way) ----
            nc.scalar.activation(d[:, :], d[:, :], Abs)

            # ---- channel sum: q = i*C + c, c-slices with (s w) fused ----
            dc = [
                strided(d[:, :], [[Q * SW, G], [C * SW, IPC], [1, SW]], c * SW)
                for c in range(C)
            ]
            s01 = s_pool.tile([128, GIW2], f16, tag="s01", name=f"s01_{di}")
            v01 = strided(s01[:, :], [[IW, G], [SW, IPC], [1, SW]])
            nc.vector.tensor_add(v01, dc[0], dc[1])
            sG = s_pool.tile([128, GIW2], f16, tag="sG", name=f"sG_{di}")
            vG = strided(sG[:, :], [[IW, G], [SW, IPC], [1, SW]])
            nc.vector.tensor_add(vG, v01, dc[2])

            # ---- running min into the shared m [i][s][w], both sub-rows
            # at once, w-restricted updates ----
            def sview(g, lo, hi):
                return strided(
                    sG[:, :], [[SW, IPC], [W, S], [1, hi - lo]], g * IW + lo
                )

            if m is None:
                # init from the first pair's center column shift: always
                # w-valid; row-invalid entries hold PAD_BIG and lose
                m = m_pool.tile([128, IW], f16, tag="m", name="m")
                nc.scalar.activation(
                    strided(m[:, :], [[SW, IPC], [W, S], [1, W]]),
                    sview(W_PAD, 0, W),
                    Copy,
                )
                order = [g for g in range(G) if g != W_PAD]
            else:
                order = list(range(G))
            for g in order:
                lo, hi = wrange(g)
                mv = strided(m[:, :], [[SW, IPC], [W, S], [1, hi - lo]], lo)
                nc.vector.tensor_tensor(mv, mv, sview(g, lo, hi), AluOpType.min)

        # ---- free-dim reduce -> [128,1] fp32 partials ----
        tot = r_pool.tile([128, 1], f32, tag="tot", name="tot")
        nc.vector.tensor_reduce(tot, m, mybir.AxisListType.X, AluOpType.add)
        nc.sync.dma_start(out_d.ap()[:, :], tot)

    nc.compile()
    return nc


def _get_nc(nh, nw):
    key = (nh, nw)
    if key not in _BUILD_CACHE:
        _BUILD_CACHE[key] = _build(nh, nw)
    return _BUILD_CACHE[key]


def _setup_trace():
    """Register the axon NTFF profile hook (the image's antenv lacks
    axon_hooks) and stub the artifact upload so trace=True works."""
    import sys
    import types

    from concourse import bass_utils

    try:
        import antenv.axon_hooks  # noqa: F401
    except ImportError:
        try:
            import trn_agent_boot.trn_boot as tb

            hook = tb._ntff_profile_via_ctypes("/opt/axon/libaxon_pjrt.so")
            mod = types.ModuleType("antenv.axon_hooks")
            mod.get_axon_ntff_profile_hook = lambda: hook
            sys.modules["antenv.axon_hooks"] = mod
        except Exception as e:  # profiling is best-effort
            print(f"ntff hook setup failed: {e}")
            return False
    bass_utils.upload_artifacts = lambda tmpdir: f"local:{tmpdir}"
    return True


def kernel(predicted, ground_truth, nh=5, nw=5):
    from concourse import bass_utils

    nh, nw = int(nh), int(nw)
    pred = np.ascontiguousarray(np.asarray(predicted, dtype=np.float32))
    gt = np.ascontiguousarray(np.asarray(ground_truth, dtype=np.float32))
    assert pred.shape == (B, C, H, W) and gt.shape == (B, C, H, W)

    nc = _get_nc(nh, nw)
    import ml_dtypes

    eye = np.concatenate(
        [np.eye(128, k=-k) for k in nc._shift_ks], axis=1
    ).astype(ml_dtypes.bfloat16)
    in_maps = [
        {
            "predicted": pred[k * IPC : (k + 1) * IPC],
            "ground_truth": gt[k * IPC : (k + 1) * IPC],
            "shifteye": eye,
        }
        for k in range(N_CORES)
    ]
    trace = bool(int(os.environ.get("NNLOSS_TRACE", "0")))
    if trace:
        trace = _setup_trace()
    res = bass_utils.run_bass_kernel_spmd(
        nc, in_maps, list(range(N_CORES)), trace=trace
    )
    LAST_EXEC_NS[0] = res.exec_time_ns
    LAST_RES[0] = res
    total = 0.0
    for r in res.results:
        total += float(np.asarray(r["partials"], dtype=np.float64).sum())
    return np.float32(total / (B * H * W))


# revision 34
# speedup vs baseline: 1.0372x; 1.0372x over previous
"""Trainium2 Bass kernel for the 5x5-neighborhood min-L1 loss (nn_NNLoss).

Computation (faithful to the reference):
    gt_pad = pad(ground_truth, rows by nw//2, cols by nh//2, value=-10000)
    norms[b,h,w,s] = sum_c |gt_pad[b,c,h+di,w+dj] - pred[b,c,h,w]|
                     for s=(di,dj), di in range(nh), dj in range(nw)
    loss = mean over (b,h,w) of min_s norms

Sharding: pure data parallel over the batch dim: 16 images -> 2 per core
across 8 NeuronCores.  Each core returns per-partition partial sums
[128,1]; the host adds them up and divides (the scalar "all-reduce").

Per-core layout (v5 -- single row-block, 2 rows per partition):
  - partition p holds image rows {2p, 2p+1} (sub-row s in {0,1}); free
    dim is [q=(img,chan), s, w].  Every HBM load is ONE dma with 2KB
    contiguous descriptors and the whole H=256 fits one partition block.
  - ground_truth is loaded ONCE via a SWDGE dma that casts f32->bf16 in
    flight (descriptor emission finishes before compute starts, so the
    Q7 ring traffic never contends with DVE); the nh row shifts
    decompose into partition shifts k in {-1,0,+1} (built from the base
    tile by two SBUF->SBUF DMAs on the otherwise-idle sync HWDGE queue)
    plus a sub-row select s'.
  - NO pad values are materialized: out-of-range column shifts are
    excluded from the running min by restricting the min-update APs to
    the valid w range, and out-of-range rows by memsetting the boundary
    partitions of the shifted tiles to +10000 (|10000 - pred| can never
    win the min: real sums are < ~30).  All memsets run on gpsimd.
  - per (di, s) unit: one wide sub (DVE, all nw column shifts via an
    overlapping-window AP at 2x bf16) -> |.| in place (ACT) -> channel
    sum (2 DVE adds) -> w-restricted running-min updates into a single
    shared m tile [i][s][w], reduced once at the end.
"""

import os

# The execution path needs the axon PJRT platform; a harness that pins
# JAX_PLATFORMS=cpu would hide the NeuronCores from jax.
if "axon" not in os.environ.get("JAX_PLATFORMS", "axon"):
    os.environ.pop("JAX_PLATFORMS", None)

import numpy as np

B, C, H, W = 16, 3, 256, 256
N_CORES = 8
IPC = B // N_CORES  # images per core
PAD_BIG = 10000.0  # stand-in for the reference's pad: never wins the min

_BUILD_CACHE = {}
LAST_EXEC_NS = [None]  # exec_time_ns of the last traced run (for test.py)
LAST_RES = [None]  # full BassKernelResults of the last run (for analysis)


def _build(nh, nw):
    """Trace the Bass/Tile program for one core. Returns the Bass object."""
    from contextlib import ExitStack

    import concourse.bacc as bacc
    import concourse.bass as bass  # noqa: F401
    import concourse.tile as tile
    from concourse import mybir
    from concourse.alu_op_type import AluOpType

    f32 = mybir.dt.float32
    # bf16, not fp16: the DVE's 2x tensor_tensor packing mode only has
    # uops for bf16 (fp16 measured at 1x on HW)
    f16 = mybir.dt.bfloat16
    Abs = mybir.ActivationFunctionType.Abs
    Copy = mybir.ActivationFunctionType.Copy

    # Faithful to the reference's crossed pad/shift pairing:
    #   row shifts   di in range(nh), offset d  = di - nw//2
    #   col shifts   g  in range(nw), offset    = g  - nh//2
    H_PAD = nw // 2
    W_PAD = nh // 2
    NDI, G = nh, nw
    S = 2  # rows packed per partition
    assert H == 128 * S
    Q = C * IPC  # fused (img, chan) chunks: 6
    SW = S * W  # 512
    FDW = Q * SW  # 3072: data columns of the packed tiles
    MARG = W_PAD  # margin columns so the window AP stays in-bounds
    GQW = G * Q * W  # 7680: one (di, s) diff tensor [g][q][w]
    GIW = G * IPC * W  # 2560: one (di, s) channel-summed tensor [g][i][w]
    IW = IPC * SW  # 1024: running-min tile [i][s][w]

    # (di, s) -> (partition shift k, source sub-row s'): the target row
    # 2p + s + (di - H_PAD) lives at partition p + k, sub-row s'
    def shift_of(di, s):
        idx = s + di - H_PAD
        return idx // S, idx % S

    units = [(di, s) for di in range(NDI) for s in range(S)]
    ks_needed = sorted({shift_of(*u)[0] for u in units})
    # process units that only need the unshifted tile first: the PE
    # builds the shifted tiles (~7us) while the first subs run
    units.sort(key=lambda u: abs(shift_of(*u)[0]))

    # valid output-w range for column shift g (shifts reading outside the
    # row are excluded from the min -- the reference's pad value loses
    # every min it enters, so exclusion is equivalent)
    def wrange(g):
        lo = max(0, W_PAD - g)
        hi = W + min(0, W_PAD - g)
        return lo, hi

    # Bacc (not raw Bass): its compile() splits multi-wait instructions
    # (TRN2 allows at most one sync wait per instruction) among other
    # required lowerings.
    nc = bacc.Bacc("TRN2", target_bir_lowering=False, debug=False)
    pred_d = nc.dram_tensor("predicted", [IPC, C, H, W], f32, kind="ExternalInput")
    gt_d = nc.dram_tensor("ground_truth", [IPC, C, H, W], f32, kind="ExternalInput")
    # stacked shifted identities [k-index, 128, 128] for the PE-based
    # partition shifts (lhsT[k, p] = 1 iff k = p + shift)
    n_eyes = len([k for k in ks_needed if k != 0])
    eye_d = nc.dram_tensor("shifteye", [128, n_eyes * 128], f16, kind="ExternalInput")
    out_d = nc.dram_tensor("partials", [128, 1], f32, kind="ExternalOutput")

    import bass_rust as _br

    def strided(ap, levels, extra_offset=0):
        """Hand-built free-dim AP on an existing [128, N] view (keeps the
        partition level and base offset)."""
        c = ap.copy()
        c.ap = _br.VecI64Pair([list(ap.ap[0])] + [list(l) for l in levels])
        if extra_offset:
            c.offset = c.offset + extra_offset
        return c

    with tile.TileContext(nc) as tc, ExitStack() as ctx:
        g_pool = ctx.enter_context(tc.tile_pool(name="gt", bufs=1))
        p_pool = ctx.enter_context(tc.tile_pool(name="pred", bufs=1))
        d_pool = ctx.enter_context(tc.tile_pool(name="d", bufs=4))
        s_pool = ctx.enter_context(tc.tile_pool(name="s", bufs=3))
        m_pool = ctx.enter_context(tc.tile_pool(name="m", bufs=1))
        r_pool = ctx.enter_context(tc.tile_pool(name="r", bufs=1))

        # ---- ground truth: one SWDGE dma, f32->bf16 cast in flight,
        # 2KB descriptors (2 contiguous rows per partition).  Both input
        # loads together read 3.1MB -- the ~9us load phase is HBM-bound
        # either way, and the in-flight cast avoids a separate cast op ----
        gt_t = {}
        gt_t[0] = g_pool.tile(
            [128, MARG + FDW + MARG], f16, tag="gt0", name="gt0"
        )
        nc.gpsimd.memset(gt_t[0][:, 0:MARG], PAD_BIG)
        nc.gpsimd.memset(gt_t[0][:, MARG + FDW :], PAD_BIG)
        nc.gpsimd.dma_start(
            gt_t[0][:, MARG : MARG + FDW].rearrange("p (q x) -> p q x", q=Q),
            gt_d.ap().rearrange("i c (p s) w -> p (i c) (s w)", s=S),
        )

        # ---- predicted: HWDGE f32 load on the ACT queue + ACT cast ----
        p_stage = p_pool.tile([128, FDW], f32, tag="p_stage", name="p_stage")
        nc.scalar.dma_start(
            p_stage.rearrange("p (q x) -> p q x", q=Q),
            pred_d.ap().rearrange("i c (p s) w -> p (i c) (s w)", s=S),
        )
        pred_t = p_pool.tile([128, FDW], f16, tag="pred", name="pred")
        nc.scalar.activation(pred_t[:, :], p_stage[:, :], Copy)

        # ---- partition-shifted gt copies, built ON-CHIP by the (idle)
        # TensorEngine: matmul with a shifted identity moves partition
        # p+k -> p exactly (bf16 x {0,1} is lossless), landing in PSUM
        # f32; one ACT op casts PSUM -> bf16 SBUF.  SBUF->SBUF DMA
        # measured 5-23 GB/s (60us+ per shift) so DMA is not an option.
        # The base tile's PAD_BIG margins shift along with the data; the
        # boundary partition (no source row) comes out 0 and is patched
        # to PAD_BIG by a tiny one-partition DMA from a const tile. ----
        WTOT = MARG + FDW + MARG
        eye_t = g_pool.tile([128, n_eyes * 128], f16, tag="eye", name="eye_t")
        nc.scalar.dma_start(eye_t[:, :], eye_d.ap())
        cpad = g_pool.tile([32, WTOT], f16, tag="cpad", name="cpad")
        nc.gpsimd.memset(cpad[:, :], PAD_BIG)
        ps_pool = ctx.enter_context(tc.tile_pool(name="ps", bufs=1, space="PSUM"))
        for ei, k in enumerate([k for k in ks_needed if k != 0]):
            t = g_pool.tile([128, WTOT], f16, tag=f"gt{k}", name=f"gt{k}")
            ps = ps_pool.tile([128, WTOT], f32, tag="ps", name=f"ps{k}")
            lhsT = eye_t[:, ei * 128 : (ei + 1) * 128]
            for c in range(0, WTOT, 512):
                wid = min(512, WTOT - c)
                nc.tensor.matmul(
                    ps[:, c : c + wid],
                    lhsT,
                    gt_t[0][:, c : c + wid],
                    start=True,
                    stop=True,
                )
            nc.scalar.activation(t[:, :], ps[:, :], Copy)
            bp = 0 if k < 0 else 127
            nc.sync.dma_start(t[bp : bp + 1, :], cpad[0:1, :])
            gt_t[k] = t
        nc._shift_ks = [k for k in ks_needed if k != 0]

        m = None
        m_init = {}
        for di, s in units:
            k, sp = shift_of(di, s)

            # ---- wide sub: all G column shifts in one 2x bf16 DVE op ----
            d = d_pool.tile([128, GQW], f16, tag="d", name=f"d{di}_{s}")
            gt_op = strided(
                gt_t[k][:, :], [[1, G], [SW, Q], [1, W]], MARG + sp * W - W_PAD
            )
            pr_op = strided(pred_t[:, :], [[0, G], [SW, Q], [1, W]], s * W)
            d_out = strided(d[:, :], [[Q * W, G], [W, Q], [1, W]])
            nc.vector.tensor_sub(d_out, gt_op, pr_op)

            # ---- |d| in place on ACT ----
            nc.scalar.activation(d[:, :], d[:, :], Abs)

            # ---- channel sum: q = i*C + c, so c-slices are strided views
            CW = C * W
            dc = [
                strided(d[:, :], [[Q * W, G], [CW, IPC], [1, W]], c * W)
                for c in range(C)
            ]
            s01 = s_pool.tile([128, GIW], f16, tag="s01", name=f"s01_{di}_{s}")
            v01 = strided(s01[:, :], [[IPC * W, G], [W, IPC], [1, W]])
            nc.vector.tensor_add(v01, dc[0], dc[1])
            sG = s_pool.tile([128, GIW], f16, tag="sG", name=f"sG_{di}_{s}")
            vG = strided(sG[:, :], [[IPC * W, G], [W, IPC], [1, W]])
            nc.vector.tensor_add(vG, v01, dc[2])

            # ---- running min into the shared m [i][s][w]: this unit's
            # s-half, w-restricted updates ----
            def sview(g, lo, hi):
                return strided(sG[:, :], [[W, IPC], [1, hi - lo]], g * IPC * W + lo)

            if m is None:
                m = m_pool.tile([128, IW], f16, tag="m", name="m")
            if s not in m_init:
                # init this s-half from the first unit's center column
                # shift: always w-valid, and row-invalid entries hold
                # PAD_BIG which later min updates displace
                m_init[s] = True
                nc.scalar.activation(
                    strided(m[:, :], [[SW, IPC], [1, W]], s * W),
                    sview(W_PAD, 0, W),
                    Copy,
                )
                order = [g for g in range(G) if g != W_PAD]
            else:
                order = list(range(G))
            for g in order:
                lo, hi = wrange(g)
                mv = strided(m[:, :], [[SW, IPC], [1, hi - lo]], s * W + lo)
                nc.vector.tensor_tensor(mv, mv, sview(g, lo, hi), AluOpType.min)

        # ---- free-dim reduce -> [128,1] fp32 partials ----
        tot = r_pool.tile([128, 1], f32, tag="tot", name="tot")
        nc.vector.tensor_reduce(tot, m, mybir.AxisListType.X, AluOpType.add)
        nc.sync.dma_start(out_d.ap()[:, :], tot)

    nc.compile()
    return nc


def _get_nc(nh, nw):
    key = (nh, nw)
    if key not in _BUILD_CACHE:
        _BUILD_CACHE[key] = _build(nh, nw)
    return _BUILD_CACHE[key]


def _setup_trace():
    """Register the axon NTFF profile hook (the image's antenv lacks
    axon_hooks) and stub the artifact upload so trace=True works."""
    import sys
    import types

    from concourse import bass_utils

    try:
        import antenv.axon_hooks  # noqa: F401
    except ImportError:
        try:
            import trn_agent_boot.trn_boot as tb

            hook = tb._ntff_profile_via_ctypes("/opt/axon/libaxon_pjrt.so")
            mod = types.ModuleType("antenv.axon_hooks")
            mod.get_axon_ntff_profile_hook = lambda: hook
            sys.modules["antenv.axon_hooks"] = mod
        except Exception as e:  # profiling is best-effort
            print(f"ntff hook setup failed: {e}")
            return False
    bass_utils.upload_artifacts = lambda tmpdir: f"local:{tmpdir}"
    return True


def kernel(predicted, ground_truth, nh=5, nw=5):
    from concourse import bass_utils

    nh, nw = int(nh), int(nw)
    pred = np.ascontiguousarray(np.asarray(predicted, dtype=np.float32))
    gt = np.ascontiguousarray(np.asarray(ground_truth, dtype=np.float32))
    assert pred.shape == (B, C, H, W) and gt.shape == (B, C, H, W)

    nc = _get_nc(nh, nw)
    import ml_dtypes

    eye = np.concatenate(
        [np.eye(128, k=-k) for k in nc._shift_ks], axis=1
    ).astype(ml_dtypes.bfloat16)
    in_maps = [
        {
            "predicted": pred[k * IPC : (k + 1) * IPC],
            "ground_truth": gt[k * IPC : (k + 1) * IPC],
            "shifteye": eye,
        }
        for k in range(N_CORES)
    ]
    trace = bool(int(os.environ.get("NNLOSS_TRACE", "0")))
    if trace:
        trace = _setup_trace()
    res = bass_utils.run_bass_kernel_spmd(
        nc, in_maps, list(range(N_CORES)), trace=trace
    )
    LAST_EXEC_NS[0] = res.exec_time_ns
    LAST_RES[0] = res
    total = 0.0
    for r in res.results:
        total += float(np.asarray(r["partials"], dtype=np.float64).sum())
    return np.float32(total / (B * H * W))


# revision 36
# speedup vs baseline: 1.0458x; 1.0083x over previous
"""Trainium2 Bass kernel for the 5x5-neighborhood min-L1 loss (nn_NNLoss).

Computation (faithful to the reference):
    gt_pad = pad(ground_truth, rows by nw//2, cols by nh//2, value=-10000)
    norms[b,h,w,s] = sum_c |gt_pad[b,c,h+di,w+dj] - pred[b,c,h,w]|
                     for s=(di,dj), di in range(nh), dj in range(nw)
    loss = mean over (b,h,w) of min_s norms

Sharding: pure data parallel over the batch dim: 16 images -> 2 per core
across 8 NeuronCores.  Each core returns per-partition partial sums
[128,1]; the host adds them up and divides (the scalar "all-reduce").

Per-core layout (v5 -- single row-block, 2 rows per partition):
  - partition p holds image rows {2p, 2p+1} (sub-row s in {0,1}); free
    dim is [q=(img,chan), s, w].  Every HBM load is ONE dma with 2KB
    contiguous descriptors and the whole H=256 fits one partition block.
  - ground_truth is loaded ONCE via a SWDGE dma that casts f32->bf16 in
    flight (descriptor emission finishes before compute starts, so the
    Q7 ring traffic never contends with DVE); the nh row shifts
    decompose into partition shifts k in {-1,0,+1} (built from the base
    tile by two SBUF->SBUF DMAs on the otherwise-idle sync HWDGE queue)
    plus a sub-row select s'.
  - NO pad values are materialized: out-of-range column shifts are
    excluded from the running min by restricting the min-update APs to
    the valid w range, and out-of-range rows by memsetting the boundary
    partitions of the shifted tiles to +10000 (|10000 - pred| can never
    win the min: real sums are < ~30).  All memsets run on gpsimd.
  - per (di, s) unit: one wide sub (DVE, all nw column shifts via an
    overlapping-window AP at 2x bf16) -> |.| in place (ACT) -> channel
    sum (2 DVE adds) -> w-restricted running-min updates into a single
    shared m tile [i][s][w], reduced once at the end.
"""

import os

# The execution path needs the axon PJRT platform; a harness that pins
# JAX_PLATFORMS=cpu would hide the NeuronCores from jax.
if "axon" not in os.environ.get("JAX_PLATFORMS", "axon"):
    os.environ.pop("JAX_PLATFORMS", None)

import numpy as np

B, C, H, W = 16, 3, 256, 256
N_CORES = 8
IPC = B // N_CORES  # images per core
PAD_BIG = 10000.0  # stand-in for the reference's pad: never wins the min

_BUILD_CACHE = {}
LAST_EXEC_NS = [None]  # exec_time_ns of the last traced run (for test.py)
LAST_RES = [None]  # full BassKernelResults of the last run (for analysis)


def _build(nh, nw):
    """Trace the Bass/Tile program for one core. Returns the Bass object."""
    from contextlib import ExitStack

    import concourse.bacc as bacc
    import concourse.bass as bass  # noqa: F401
    import concourse.tile as tile
    from concourse import mybir
    from concourse.alu_op_type import AluOpType

    f32 = mybir.dt.float32
    # bf16, not fp16: the DVE's 2x tensor_tensor packing mode only has
    # uops for bf16 (fp16 measured at 1x on HW)
    f16 = mybir.dt.bfloat16
    Abs = mybir.ActivationFunctionType.Abs
    Copy = mybir.ActivationFunctionType.Copy

    # Faithful to the reference's crossed pad/shift pairing:
    #   row shifts   di in range(nh), offset d  = di - nw//2
    #   col shifts   g  in range(nw), offset    = g  - nh//2
    H_PAD = nw // 2
    W_PAD = nh // 2
    NDI, G = nh, nw
    S = 2  # rows packed per partition
    assert H == 128 * S
    Q = C * IPC  # fused (img, chan) chunks: 6
    SW = S * W  # 512
    FDW = Q * SW  # 3072: data columns of the packed tiles
    MARG = W_PAD  # margin columns so the window AP stays in-bounds
    GQW = G * Q * W  # 7680: one (di, s) diff tensor [g][q][w]
    GIW = G * IPC * W  # 2560: one (di, s) channel-summed tensor [g][i][w]
    IW = IPC * SW  # 1024: running-min tile [i][s][w]

    # (di, s) -> (partition shift k, source sub-row s'): the target row
    # 2p + s + (di - H_PAD) lives at partition p + k, sub-row s'
    def shift_of(di, s):
        idx = s + di - H_PAD
        return idx // S, idx % S

    units = [(di, s) for di in range(NDI) for s in range(S)]
    ks_needed = sorted({shift_of(*u)[0] for u in units})
    # process units that only need the unshifted tile first: the PE
    # builds the shifted tiles (~7us) while the first subs run
    units.sort(key=lambda u: abs(shift_of(*u)[0]))

    # valid output-w range for column shift g (shifts reading outside the
    # row are excluded from the min -- the reference's pad value loses
    # every min it enters, so exclusion is equivalent)
    def wrange(g):
        lo = max(0, W_PAD - g)
        hi = W + min(0, W_PAD - g)
        return lo, hi

    # Bacc (not raw Bass): its compile() splits multi-wait instructions
    # (TRN2 allows at most one sync wait per instruction) among other
    # required lowerings.
    nc = bacc.Bacc("TRN2", target_bir_lowering=False, debug=False)
    pred_d = nc.dram_tensor("predicted", [IPC, C, H, W], f32, kind="ExternalInput")
    gt_d = nc.dram_tensor("ground_truth", [IPC, C, H, W], f32, kind="ExternalInput")
    # stacked shifted identities [k-index, 128, 128] for the PE-based
    # partition shifts (lhsT[k, p] = 1 iff k = p + shift)
    n_eyes = len([k for k in ks_needed if k != 0])
    eye_d = nc.dram_tensor("shifteye", [128, n_eyes * 128], f16, kind="ExternalInput")
    out_d = nc.dram_tensor("partials", [128, 1], f32, kind="ExternalOutput")

    import bass_rust as _br

    def strided(ap, levels, extra_offset=0):
        """Hand-built free-dim AP on an existing [128, N] view (keeps the
        partition level and base offset)."""
        c = ap.copy()
        c.ap = _br.VecI64Pair([list(ap.ap[0])] + [list(l) for l in levels])
        if extra_offset:
            c.offset = c.offset + extra_offset
        return c

    with tile.TileContext(nc) as tc, ExitStack() as ctx:
        g_pool = ctx.enter_context(tc.tile_pool(name="gt", bufs=1))
        p_pool = ctx.enter_context(tc.tile_pool(name="pred", bufs=1))
        d_pool = ctx.enter_context(tc.tile_pool(name="d", bufs=4))
        s_pool = ctx.enter_context(tc.tile_pool(name="s", bufs=3))
        m_pool = ctx.enter_context(tc.tile_pool(name="m", bufs=1))
        r_pool = ctx.enter_context(tc.tile_pool(name="r", bufs=1))

        # ---- ground truth: one SWDGE dma, f32->bf16 cast in flight,
        # 2KB descriptors (2 contiguous rows per partition).  Both input
        # loads together read 3.1MB -- the ~9us load phase is HBM-bound
        # either way, and the in-flight cast avoids a separate cast op ----
        gt_t = {}
        gt_t[0] = g_pool.tile(
            [128, MARG + FDW + MARG], f16, tag="gt0", name="gt0"
        )
        nc.gpsimd.memset(gt_t[0][:, 0:MARG], PAD_BIG)
        nc.gpsimd.memset(gt_t[0][:, MARG + FDW :], PAD_BIG)
        # two dmas, one per image: the first image's half lands ~4us
        # earlier, letting the leading units' image-0 sub halves start
        HF = C * SW  # columns per image in the packed free dim
        for i in range(IPC):
            nc.gpsimd.dma_start(
                gt_t[0][:, MARG + i * HF : MARG + (i + 1) * HF].rearrange(
                    "p (q x) -> p q x", q=C
                ),
                gt_d.ap()[i : i + 1].rearrange("i c (p s) w -> p (i c) (s w)", s=S),
            )

        # ---- predicted: HWDGE f32 load on the ACT queue + ACT cast ----
        p_stage = p_pool.tile([128, FDW], f32, tag="p_stage", name="p_stage")
        nc.scalar.dma_start(
            p_stage.rearrange("p (q x) -> p q x", q=Q),
            pred_d.ap().rearrange("i c (p s) w -> p (i c) (s w)", s=S),
        )
        pred_t = p_pool.tile([128, FDW], f16, tag="pred", name="pred")
        nc.scalar.activation(pred_t[:, :], p_stage[:, :], Copy)

        # ---- partition-shifted gt copies, built ON-CHIP by the (idle)
        # TensorEngine: matmul with a shifted identity moves partition
        # p+k -> p exactly (bf16 x {0,1} is lossless), landing in PSUM
        # f32; one ACT op casts PSUM -> bf16 SBUF.  SBUF->SBUF DMA
        # measured 5-23 GB/s (60us+ per shift) so DMA is not an option.
        # The base tile's PAD_BIG margins shift along with the data; the
        # boundary partition (no source row) comes out 0 and is patched
        # to PAD_BIG by a tiny one-partition DMA from a const tile. ----
        WTOT = MARG + FDW + MARG
        eye_t = g_pool.tile([128, n_eyes * 128], f16, tag="eye", name="eye_t")
        nc.scalar.dma_start(eye_t[:, :], eye_d.ap())
        cpad = g_pool.tile([32, WTOT], f16, tag="cpad", name="cpad")
        nc.gpsimd.memset(cpad[:, :], PAD_BIG)
        ps_pool = ctx.enter_context(tc.tile_pool(name="ps", bufs=1, space="PSUM"))
        for ei, k in enumerate([k for k in ks_needed if k != 0]):
            t = g_pool.tile([128, WTOT], f16, tag=f"gt{k}", name=f"gt{k}")
            ps = ps_pool.tile([128, WTOT], f32, tag="ps", name=f"ps{k}")
            lhsT = eye_t[:, ei * 128 : (ei + 1) * 128]
            for c in range(0, WTOT, 512):
                wid = min(512, WTOT - c)
                nc.tensor.matmul(
                    ps[:, c : c + wid],
                    lhsT,
                    gt_t[0][:, c : c + wid],
                    start=True,
                    stop=True,
                )
            nc.scalar.activation(t[:, :], ps[:, :], Copy)
            bp = 0 if k < 0 else 127
            nc.sync.dma_start(t[bp : bp + 1, :], cpad[0:1, :])
            gt_t[k] = t
        nc._shift_ks = [k for k in ks_needed if k != 0]

        m = None
        m_init = {}
        for ui, (di, s) in enumerate(units):
            k, sp = shift_of(di, s)

            # ---- wide sub: all G column shifts in one 2x bf16 DVE op.
            # The first two units sub per-image so the image-0 half can
            # start as soon as the first base-load dma lands. ----
            d = d_pool.tile([128, GQW], f16, tag="d", name=f"d{di}_{s}")
            isplit = [(0, C), (C, Q)] if ui < 2 else [(0, Q)]
            for q0, q1 in isplit:
                nq = q1 - q0
                gt_op = strided(
                    gt_t[k][:, :],
                    [[1, G], [SW, nq], [1, W]],
                    MARG + sp * W - W_PAD + q0 * SW,
                )
                pr_op = strided(
                    pred_t[:, :], [[0, G], [SW, nq], [1, W]], s * W + q0 * SW
                )
                d_out = strided(
                    d[:, :], [[Q * W, G], [W, nq], [1, W]], q0 * W
                )
                nc.vector.tensor_sub(d_out, gt_op, pr_op)

            # ---- |d| in place on ACT ----
            nc.scalar.activation(d[:, :], d[:, :], Abs)

            # ---- channel sum: q = i*C + c, so c-slices are strided views
            CW = C * W
            dc = [
                strided(d[:, :], [[Q * W, G], [CW, IPC], [1, W]], c * W)
                for c in range(C)
            ]
            s01 = s_pool.tile([128, GIW], f16, tag="s01", name=f"s01_{di}_{s}")
            v01 = strided(s01[:, :], [[IPC * W, G], [W, IPC], [1, W]])
            nc.vector.tensor_add(v01, dc[0], dc[1])
            sG = s_pool.tile([128, GIW], f16, tag="sG", name=f"sG_{di}_{s}")
            vG = strided(sG[:, :], [[IPC * W, G], [W, IPC], [1, W]])
            nc.vector.tensor_add(vG, v01, dc[2])

            # ---- running min into the shared m [i][s][w]: this unit's
            # s-half, w-restricted updates ----
            def sview(g, lo, hi):
                return strided(sG[:, :], [[W, IPC], [1, hi - lo]], g * IPC * W + lo)

            if m is None:
                m = m_pool.tile([128, IW], f16, tag="m", name="m")
            if s not in m_init:
                # init this s-half from the first unit's center column
                # shift: always w-valid, and row-invalid entries hold
                # PAD_BIG which later min updates displace
                m_init[s] = True
                nc.scalar.activation(
                    strided(m[:, :], [[SW, IPC], [1, W]], s * W),
                    sview(W_PAD, 0, W),
                    Copy,
                )
                order = [g for g in range(G) if g != W_PAD]
            else:
                order = list(range(G))
            for g in order:
                lo, hi = wrange(g)
                mv = strided(m[:, :], [[SW, IPC], [1, hi - lo]], s * W + lo)
                nc.vector.tensor_tensor(mv, mv, sview(g, lo, hi), AluOpType.min)

        # ---- free-dim reduce -> [128,1] fp32 partials ----
        tot = r_pool.tile([128, 1], f32, tag="tot", name="tot")
        nc.vector.tensor_reduce(tot, m, mybir.AxisListType.X, AluOpType.add)
        nc.sync.dma_start(out_d.ap()[:, :], tot)

    nc.compile()
    return nc


def _get_nc(nh, nw):
    key = (nh, nw)
    if key not in _BUILD_CACHE:
        _BUILD_CACHE[key] = _build(nh, nw)
    return _BUILD_CACHE[key]


def _setup_trace():
    """Register the axon NTFF profile hook (the image's antenv lacks
    axon_hooks) and stub the artifact upload so trace=True works."""
    import sys
    import types

    from concourse import bass_utils

    try:
        import antenv.axon_hooks  # noqa: F401
    except ImportError:
        try:
            import trn_agent_boot.trn_boot as tb

            hook = tb._ntff_profile_via_ctypes("/opt/axon/libaxon_pjrt.so")
            mod = types.ModuleType("antenv.axon_hooks")
            mod.get_axon_ntff_profile_hook = lambda: hook
            sys.modules["antenv.axon_hooks"] = mod
        except Exception as e:  # profiling is best-effort
            print(f"ntff hook setup failed: {e}")
            return False
    bass_utils.upload_artifacts = lambda tmpdir: f"local:{tmpdir}"
    return True


def kernel(predicted, ground_truth, nh=5, nw=5):
    from concourse import bass_utils

    nh, nw = int(nh), int(nw)
    pred = np.ascontiguousarray(np.asarray(predicted, dtype=np.float32))
    gt = np.ascontiguousarray(np.asarray(ground_truth, dtype=np.float32))
    assert pred.shape == (B, C, H, W) and gt.shape == (B, C, H, W)

    nc = _get_nc(nh, nw)
    import ml_dtypes

    eye = np.concatenate(
        [np.eye(128, k=-k) for k in nc._shift_ks], axis=1
    ).astype(ml_dtypes.bfloat16)
    in_maps = [
        {
            "predicted": pred[k * IPC : (k + 1) * IPC],
            "ground_truth": gt[k * IPC : (k + 1) * IPC],
            "shifteye": eye,
        }
        for k in range(N_CORES)
    ]
    trace = bool(int(os.environ.get("NNLOSS_TRACE", "0")))
    if trace:
        trace = _setup_trace()
    res = bass_utils.run_bass_kernel_spmd(
        nc, in_maps, list(range(N_CORES)), trace=trace
    )
    LAST_EXEC_NS[0] = res.exec_time_ns
    LAST_RES[0] = res
    total = 0.0
    for r in res.results:
        total += float(np.asarray(r["partials"], dtype=np.float64).sum())
    return np.float32(total / (B * H * W))
